# revision 33
# baseline (speedup 1.0000x reference)
"""LoRA attention kernel for 8 Trainium2 NeuronCores.

Sharding: data-parallel over batch B=2 (cores 0-3 -> b=0, cores 4-7 -> b=1),
tensor-parallel over heads within each batch group (4 heads/core). LoRA paths
and q/v base linears are folded host-side into one effective qkv weight.

Key optimizations over the fp32r baseline:
- All matmuls in bf16: fp32r never registers activity with the PE's HAM
  clock gate, so the array sat at K=4/8 (1.2 GHz) for ~485us of the run and
  paid full-rate LDWEIGHTS (no FWL). bf16 runs warm at 2.4 GHz with fast
  weight load.
- Key-padding mask applied by gathering valid tokens host-side: k/v
  projections, scores, exp and P@V run over ~Nv~1024 instead of 2048 keys.
- Softmax denominator via an augmented ones-column in the P@V matmul;
  reciprocal via the fast custom DVE op on [1,N) + gpsimd partition
  broadcast (baseline burned 52us in single-lane DVE reciprocals plus a
  DRAM round-trip broadcast).
- Per-head bf16 AllGathers of attention outputs overlap the next head's
  compute (baseline: one 103us fp32 AllGather dead on the tail).
"""

import sys
from contextlib import ExitStack

import numpy as np
import ml_dtypes

for _p in ("/opt/trn_rl_repo", "/opt/trn_rl_repo/concourse"):
    if _p not in sys.path:
        sys.path.insert(0, _p)

import concourse.bass as bass
import concourse.mybir as mybir
import concourse.tile as tile
from concourse import bacc
from concourse import bass_utils
from concourse import library_config
from concourse.masks import make_identity

F32 = mybir.dt.float32
BF16 = mybir.dt.bfloat16
EXP = mybir.ActivationFunctionType.Exp
BF = ml_dtypes.bfloat16

H, D, DIM, R = 16, 64, 1024, 10
B, N = 2, 2048
NCORES = 8
GROUPS = [[0, 1, 2, 3], [4, 5, 6, 7]]
HPC = H // 4          # heads per core
HD = HPC * D          # 256 qkv rows per core per projection
ATT = float(D) ** -0.5
LS = 1.0 / R

KT = DIM // 128       # 8 contraction tiles
NCH = N // 512        # 4 moving chunks of 512

# test harness hooks
TRACE = False
TRACE_DIR = None
LAST_RESULTS = None

_NC_CACHE = {}


def _build_nc(NV):
    """Build the SPMD program for NV (padded valid-key count, mult of 128)."""
    NVT = NV // 128                      # m-tiles over valid keys
    nc = bacc.Bacc(None, target_bir_lowering=False, num_devices=NCORES)

    xT = nc.dram_tensor("xT", (DIM, N), BF16, kind="ExternalInput")
    xvT = nc.dram_tensor("xvT", (DIM, NV), BF16, kind="ExternalInput")
    wT = nc.dram_tensor("wT", (DIM, 3 * HD), BF16, kind="ExternalInput")
    pb = nc.dram_tensor("pb", (3 * HD,), F32, kind="ExternalInput")
    aug = nc.dram_tensor("aug", (NV,), F32, kind="ExternalInput")
    woT = nc.dram_tensor("woT", (DIM, HD), BF16, kind="ExternalInput")
    bo = nc.dram_tensor("bo", (HD,), F32, kind="ExternalInput")
    outT = nc.dram_tensor("outT", (HD, N), BF16, kind="ExternalOutput")

    # two collectives: heads 0-1 gathered mid-attention, heads 2-3 at the
    # end (CC firmware costs ~9us per op + skew, so fewer ops win)
    agin = [nc.dram_tensor(f"agin{p}", (128, N), BF16) for p in range(2)]
    agout = [nc.dram_tensor(f"agout{p}", (512, N), BF16) for p in range(2)]

    with ExitStack() as ctx:
        tc = ctx.enter_context(tile.TileContext(nc))
        const = ctx.enter_context(tc.tile_pool(name="const", bufs=1))

        nc.gpsimd.load_library(library_config.attn)

        ident_f32 = const.tile([128, 128], F32)
        make_identity(nc, ident_f32)
        ident = const.tile([128, 128], BF16)
        nc.vector.tensor_copy(ident, ident_f32)

        pb_sb = const.tile([128, 6], F32)
        nc.sync.dma_start(out=pb_sb, in_=pb[:].rearrange("(i p) -> p i", p=128))
        aug_sb = const.tile([128, NVT], F32)
        nc.sync.dma_start(out=aug_sb, in_=aug[:].rearrange("(t p) -> p t", p=128))
        bo_sb = const.tile([128, HD // 128], F32)
        woT_sb = const.tile([128, KT, HD], BF16)
        woT_r = woT[:, :].rearrange("(k p) c -> p k c", p=128)

        qkvT = const.tile([128, 4, N], BF16)            # [q0 q1 | v0 v1]
        # k stationaries padded to 128 rows (zeros kill the paired head's q
        # contribution) and v stationaries padded to 128 cols — both enable
        # the PE fast-weight-load path (needs 128-row/col stationary).
        kTp = const.tile([128, HPC, NV], BF16)
        vsb = const.tile([128, NVT, HPC, 128], BF16)    # v[m,d] | aug | zeros
        att_sb = const.tile([128, 2 * HPC, N], BF16)    # gathered heads

        # warm the PE clock gate while input DMAs land (results unused);
        # 512-wide matmuls give a stronger activity signal than transposes
        wrhs = const.tile([128, 512], BF16)
        nc.vector.memset(wrhs, 0.0)
        with tc.tile_pool(name="pp_w", bufs=1, space="PSUM") as ppw:
            wt = ppw.tile([128, 512], F32, tag="warm")
            for _ in range(56):
                nc.tensor.matmul(wt, lhsT=ident, rhs=wrhs,
                                 start=True, stop=True)

        # ---- phase 1: fused qkv projection + v transpose/mask ----
        with tc.tile_pool(name="xw", bufs=1) as xw, \
             tc.tile_pool(name="pp_proj", bufs=6, space="PSUM") as ppp, \
             tc.tile_pool(name="pp_vt", bufs=2, space="PSUM") as ppvt:
            wT_sb = xw.tile([128, KT, 3 * HD], BF16)
            xT_sb = xw.tile([128, KT, N], BF16)
            xvT_sb = xw.tile([128, KT, NV], BF16)
            wT_r = wT[:, :].rearrange("(k p) m -> p k m", p=128)
            xT_r = xT[:, :].rearrange("(k p) n -> p k n", p=128)
            xvT_r = xvT[:, :].rearrange("(k p) n -> p k n", p=128)
            # DMA order matches consumption: wT+xT half 0 (q first half),
            # then xT half 1, then xvT (k/v), then late consts
            for k in range(KT):
                nc.sync.dma_start(out=wT_sb[:, k, :], in_=wT_r[:, k, :])
                nc.sync.dma_start(out=xT_sb[:, k, 0:N // 2],
                                  in_=xT_r[:, k, 0:N // 2])
            for k in range(KT):
                nc.sync.dma_start(out=xT_sb[:, k, N // 2:N],
                                  in_=xT_r[:, k, N // 2:N])
            for k in range(KT):
                nc.sync.dma_start(out=xvT_sb[:, k, :], in_=xvT_r[:, k, :])
            nc.sync.dma_start(out=bo_sb,
                              in_=bo[:].rearrange("(c p) -> p c", p=128))
            for k in range(KT):
                nc.sync.dma_start(out=woT_sb[:, k, :], in_=woT_r[:, k, :])

            # q rowtiles (i=0,1), k-outer per n-half: accumulation is paced
            # by the xT DMA stream instead of waiting for the last ktile
            for half in range(2):
                pss = [ppp.tile([128, 512], F32, tag="ps",
                                name=f"psq{half}_{c}") for c in range(4)]
                for k in range(KT):
                    for i in range(2):
                        lhs = wT_sb[:, k, i * 128:(i + 1) * 128]
                        for cc in range(2):
                            c = half * 2 + cc
                            nc.tensor.matmul(
                                pss[i * 2 + cc], lhsT=lhs,
                                rhs=xT_sb[:, k, c * 512:(c + 1) * 512],
                                start=(k == 0), stop=(k == KT - 1),
                            )
                for i in range(2):
                    for cc in range(2):
                        c = half * 2 + cc
                        nc.vector.tensor_scalar_add(
                            qkvT[:, i, c * 512:(c + 1) * 512],
                            pss[i * 2 + cc], pb_sb[:, i:i + 1],
                        )

            # zero pads once (disjoint partition/col ranges from the writes).
            # head h's k rows sit in the partition half matching its q rows
            # (h%2), the other half is zeros.
            for h in range(HPC):
                z0 = 64 if h % 2 == 0 else 0
                nc.vector.memset(kTp[z0:z0 + 64, h, :], 0.0)
            nc.vector.memset(vsb[:, :, :, D + 1:128], 0.0)

            # k/v rowtile pairs over NV gathered tokens, k-outer
            vchunks = [(s, min(512, NV - s)) for s in range(0, NV, 512)]
            for pair in range(2):       # 0: k rowtiles (2,3), 1: v (4,5)
                pss = [ppp.tile([128, 512], F32, tag="ps",
                                name=f"pskv{pair}_{t}")
                       for t in range(2 * len(vchunks))]
                for k in range(KT):
                    for j in range(2):
                        i = 2 + pair * 2 + j
                        lhs = wT_sb[:, k, i * 128:(i + 1) * 128]
                        for c, (s, w) in enumerate(vchunks):
                            nc.tensor.matmul(
                                pss[j * len(vchunks) + c][:, 0:w], lhsT=lhs,
                                rhs=xvT_sb[:, k, s:s + w],
                                start=(k == 0), stop=(k == KT - 1),
                            )
                for j in range(2):
                    i = 2 + pair * 2 + j
                    for c, (s, w) in enumerate(vchunks):
                        ps = pss[j * len(vchunks) + c]
                        if pair == 0:
                            # k rows (no bias) into kTp, each head's rows in
                            # its own partition half
                            for jj in range(2):
                                nc.vector.tensor_copy(
                                    kTp[jj * 64:jj * 64 + 64, 2 * j + jj,
                                        s:s + w],
                                    ps[jj * 64:jj * 64 + 64, 0:w],
                                )
                        else:
                            nc.vector.tensor_scalar_add(
                                qkvT[:, 2 + j, s:s + w], ps[:, 0:w],
                                pb_sb[:, i:i + 1],
                            )

            # transpose vT -> v[m, d]; zero pad rows; aug ones column
            for vi in range(2):
                for t in range(NVT):
                    vt = ppvt.tile([128, 128], BF16, tag="vt",
                                   name=f"vt{vi}_{t}")
                    nc.tensor.transpose(
                        vt, qkvT[:, 2 + vi, t * 128:(t + 1) * 128], ident
                    )
                    for hh in range(2):
                        h = vi * 2 + hh
                        nc.vector.tensor_scalar_mul(
                            vsb[:, t, h, 0:D], vt[:, hh * 64:hh * 64 + 64],
                            aug_sb[:, t:t + 1],
                        )
            for h in range(HPC):
                for t in range(NVT):
                    nc.vector.tensor_copy(vsb[:, t, h, D:D + 1],
                                          aug_sb[:, t:t + 1])

        # ---- phase 2: attention per head, AllGather overlapped ----
        with tc.tile_pool(name="expool", bufs=4) as expool, \
             tc.tile_pool(name="attp", bufs=2) as attp, \
             tc.tile_pool(name="ocpp", bufs=2) as ocpp, \
             tc.tile_pool(name="dnp", bufs=2) as dnp, \
             tc.tile_pool(name="recp", bufs=2) as recp, \
             tc.tile_pool(name="recbp", bufs=2) as recbp, \
             tc.tile_pool(name="pp_o", bufs=1, space="PSUM") as ppo, \
             tc.tile_pool(name="pp_st", bufs=2, space="PSUM") as ppst:
            for lh in range(HPC):
                ih = lh // 2
                qTh = qkvT[:, ih, :]
                op = ppo.tile([128, N], F32, tag="op")
                for t in range(NVT):
                    lhs = kTp[:, lh, t * 128:(t + 1) * 128]
                    for nh in range(2):
                        st = ppst.tile([128, 1024], F32, tag="st",
                                       name=f"st{lh}_{t}_{nh}")
                        for cc in range(2):
                            nc.tensor.matmul(
                                st[:, cc * 512:(cc + 1) * 512], lhsT=lhs,
                                rhs=qTh[:, nh * 1024 + cc * 512:
                                        nh * 1024 + (cc + 1) * 512],
                                start=True, stop=True,
                            )
                        ex = expool.tile([128, 1024], BF16, tag="ex",
                                         name=f"ex{lh}_{t}_{nh}")
                        nc.scalar.activation(ex, st, EXP)
                        vlhs = vsb[:, t, lh, :]
                        for cc in range(2):
                            nc.tensor.matmul(
                                op[:, nh * 1024 + cc * 512:
                                   nh * 1024 + (cc + 1) * 512],
                                lhsT=vlhs,
                                rhs=ex[:, cc * 512:(cc + 1) * 512],
                                start=(t == 0), stop=(t == NVT - 1),
                            )
                # drain PSUM to SBUF first (ALL op readers up front so the
                # next head's PV gets the banks ASAP), then normalize per
                # n-half so the chain pipelines
                ocps, dns = [], []
                for nh in range(2):
                    sl = slice(nh * (N // 2), (nh + 1) * (N // 2))
                    ocp = ocpp.tile([D, N // 2], F32, tag="ocp",
                                    name=f"ocp{lh}_{nh}")
                    nc.vector.tensor_copy(ocp, op[0:D, sl])
                    dn = dnp.tile([1, N // 2], F32, tag="dn",
                                  name=f"dn{lh}_{nh}")
                    nc.vector.tensor_copy(dn, op[D:D + 1, sl])
                    ocps.append(ocp)
                    dns.append(dn)
                for nh in range(2):
                    sl = slice(nh * (N // 2), (nh + 1) * (N // 2))
                    rec = recp.tile([1, N // 2], F32, tag="rec",
                                    name=f"rec{lh}_{nh}")
                    nc.vector.reciprocal_approx_fast(out=rec, in_=dns[nh])
                    recb = recbp.tile([64, N // 2], F32, tag="recb",
                                      name=f"recb{lh}_{nh}")
                    nc.gpsimd.partition_broadcast(recb, rec)
                    att = attp.tile([64, N // 2], BF16, tag="att",
                                    name=f"att{lh}_{nh}")
                    nc.vector.tensor_mul(att, ocps[nh], recb)
                    pg = lh // 2
                    row = (lh % 2) * 64
                    nc.sync.dma_start(out=agin[pg][row:row + 64, sl], in_=att)
                if lh % 2 == 1:
                    pg = lh // 2
                    nc.gpsimd.collective_compute(
                        "AllGather",
                        mybir.AluOpType.bypass,
                        replica_groups=GROUPS,
                        ins=[agin[pg][:, :].opt()],
                        outs=[agout[pg][:, :].opt()],
                    )
                    for r in range(4):
                        nc.sync.dma_start(
                            out=att_sb[:, 4 * pg + r, :],
                            in_=agout[pg][r * 128:(r + 1) * 128, :])

        # ---- phase 3: output projection slice ----
        with tc.tile_pool(name="outp", bufs=2) as outp, \
             tc.tile_pool(name="pp_f", bufs=2, space="PSUM") as ppf:
            out_r = outT[:, :].rearrange("(c p) n -> p c n", p=128)
            fps = [ppf.tile([128, N], F32, tag="fp", name=f"fp{ct}")
                   for ct in range(2)]
            # kt 0-3 (first AllGather) accumulate early
            for kt in range(4):
                for ct in range(2):
                    lhs = woT_sb[:, kt, ct * 128:(ct + 1) * 128]
                    for c in range(NCH):
                        nc.tensor.matmul(
                            fps[ct][:, c * 512:(c + 1) * 512],
                            lhsT=lhs,
                            rhs=att_sb[:, kt, c * 512:(c + 1) * 512],
                            start=(kt == 0), stop=False,
                        )
            # kt 4-7 (second AllGather) finished per (ct, nh) so each
            # bias-add + store leaves as soon as its chunks are done
            for ct in range(2):
                for nh in range(2):
                    for cc in range(2):
                        c = nh * 2 + cc
                        for kt in range(4, KT):
                            nc.tensor.matmul(
                                fps[ct][:, c * 512:(c + 1) * 512],
                                lhsT=woT_sb[:, kt, ct * 128:(ct + 1) * 128],
                                rhs=att_sb[:, kt, c * 512:(c + 1) * 512],
                                start=False, stop=(kt == KT - 1),
                            )
                    sl = slice(nh * (N // 2), (nh + 1) * (N // 2))
                    ot = outp.tile([128, N // 2], BF16, tag="ot",
                                   name=f"ot{ct}_{nh}")
                    nc.vector.tensor_scalar_add(ot, fps[ct][:, sl],
                                                bo_sb[:, ct:ct + 1])
                    nc.sync.dma_start(out=out_r[:, ct, sl], in_=ot)

    nc.finalize()
    return nc


def _prep_core_inputs(inputs, c, idxs, NV):
    b, g = c // 4, c % 4
    rows = slice(g * HD, (g + 1) * HD)
    w_qkv = np.asarray(inputs["w_qkv"], np.float32)
    Wq = (w_qkv[0:H * D][rows]
          + np.asarray(inputs["wq_base"], np.float32)[rows]
          + LS * (np.asarray(inputs["wq_B"], np.float32)[rows]
                  @ np.asarray(inputs["wq_A"], np.float32))) * ATT
    Wk = w_qkv[H * D:2 * H * D][rows]
    Wv = (w_qkv[2 * H * D:3 * H * D][rows]
          + np.asarray(inputs["wv_base"], np.float32)[rows]
          + LS * (np.asarray(inputs["wv_B"], np.float32)[rows]
                  @ np.asarray(inputs["wv_A"], np.float32)))
    wTv = np.ascontiguousarray(
        np.concatenate([Wq, Wk, Wv], 0).T.astype(BF))
    pbv = np.concatenate([
        np.asarray(inputs["bq_base"], np.float32)[rows] * ATT,
        np.zeros(HD, np.float32),
        np.asarray(inputs["bv_base"], np.float32)[rows],
    ]).astype(np.float32)

    x = np.asarray(inputs["x"], np.float32)[b]            # [N, DIM]
    idx = idxs[b]
    nv = len(idx)
    xv = np.zeros((NV, DIM), np.float32)
    xv[:nv] = x[idx]
    augv = np.zeros(NV, np.float32)
    augv[:nv] = 1.0
    xTv = np.ascontiguousarray(x.T.astype(BF))
    xvTv = np.ascontiguousarray(xv.T.astype(BF))

    # out-proj columns permuted to match the two AllGathers' row order:
    # kt 0-3 block r = rank r's heads (4r+0, 4r+1); kt 4-7 = (4r+2, 4r+3)
    perm = np.empty(H * D, np.int64)
    for i in range(H * D):
        kt, r = divmod(i, 2 * D)
        jj, d = divmod(r, D)
        if kt < 4:
            h = 4 * kt + jj
        else:
            h = 4 * (kt - 4) + 2 + jj
        perm[i] = h * D + d
    w_out = np.asarray(inputs["w_out"], np.float32)
    woTv = np.ascontiguousarray(w_out[rows, :][:, perm].T.astype(BF))
    bov = np.asarray(inputs["b_out"], np.float32)[rows]
    return {"xT": xTv, "xvT": xvTv, "wT": wTv, "pb": pbv, "aug": augv,
            "woT": woTv, "bo": bov}


def kernel(**inputs):
    global LAST_RESULTS
    mask = np.asarray(inputs["mask"]).astype(bool)
    idxs = [np.flatnonzero(mask[b]) for b in range(B)]
    NV = max(128, -(-max(len(ix) for ix in idxs) // 128) * 128)
    if NV not in _NC_CACHE:
        _NC_CACHE[NV] = _build_nc(NV)
    nc = _NC_CACHE[NV]
    in_maps = [_prep_core_inputs(inputs, c, idxs, NV) for c in range(NCORES)]
    res = bass_utils.run_bass_kernel_spmd(
        nc, in_maps, core_ids=list(range(NCORES)),
        trace=TRACE, tmpdir=TRACE_DIR,
    )
    LAST_RESULTS = res
    out = np.empty((B, N, DIM), np.float32)
    for c in range(NCORES):
        b, g = c // 4, c % 4
        out[b, :, g * HD:(g + 1) * HD] = \
            np.asarray(res.results[c]["outT"]).astype(np.float32).T
    return out


# revision 37
# speedup vs baseline: 1.1569x; 1.1569x over previous
"""LoRA attention kernel for 8 Trainium2 NeuronCores.

Sharding: data-parallel over batch B=2 (cores 0-3 -> b=0, cores 4-7 -> b=1),
tensor-parallel over heads within each batch group (4 heads/core). LoRA paths
and q/v base linears are folded host-side into one effective qkv weight.

Key optimizations over the fp32r baseline:
- All matmuls in bf16: fp32r never registers activity with the PE's HAM
  clock gate, so the array sat at K=4/8 (1.2 GHz) for ~485us of the run and
  paid full-rate LDWEIGHTS (no FWL). bf16 runs warm at 2.4 GHz with fast
  weight load.
- Key-padding mask applied by gathering valid tokens host-side: k/v
  projections, scores, exp and P@V run over ~Nv~1024 instead of 2048 keys.
- Softmax denominator via an augmented ones-column in the P@V matmul;
  reciprocal via the fast custom DVE op on [1,N) + gpsimd partition
  broadcast (baseline burned 52us in single-lane DVE reciprocals plus a
  DRAM round-trip broadcast).
- Per-head bf16 AllGathers of attention outputs overlap the next head's
  compute (baseline: one 103us fp32 AllGather dead on the tail).
"""

import sys
from contextlib import ExitStack

import numpy as np
import ml_dtypes

for _p in ("/opt/trn_rl_repo", "/opt/trn_rl_repo/concourse"):
    if _p not in sys.path:
        sys.path.insert(0, _p)

import concourse.bass as bass
import concourse.mybir as mybir
import concourse.tile as tile
from concourse import bacc
from concourse import bass_utils
from concourse import library_config
from concourse.masks import make_identity

F32 = mybir.dt.float32
BF16 = mybir.dt.bfloat16
EXP = mybir.ActivationFunctionType.Exp
BF = ml_dtypes.bfloat16

H, D, DIM, R = 16, 64, 1024, 10
B, N = 2, 2048
NCORES = 8
GROUPS = [[0, 1, 2, 3], [4, 5, 6, 7]]
HPC = H // 4          # heads per core
HD = HPC * D          # 256 qkv rows per core per projection
ATT = float(D) ** -0.5
LS = 1.0 / R

KT = DIM // 128       # 8 contraction tiles
NCH = N // 512        # 4 moving chunks of 512

# test harness hooks
TRACE = False
TRACE_DIR = None
LAST_RESULTS = None

_NC_CACHE = {}


def _build_nc(NV):
    """Build the SPMD program for NV (padded valid-key count, mult of 128)."""
    NVT = NV // 128                      # m-tiles over valid keys
    nc = bacc.Bacc(None, target_bir_lowering=False, num_devices=NCORES)

    xT = nc.dram_tensor("xT", (DIM, N), BF16, kind="ExternalInput")
    xvT = nc.dram_tensor("xvT", (DIM, NV), BF16, kind="ExternalInput")
    wT = nc.dram_tensor("wT", (DIM, 3 * HD), BF16, kind="ExternalInput")
    pb = nc.dram_tensor("pb", (3 * HD,), F32, kind="ExternalInput")
    aug = nc.dram_tensor("aug", (NV,), F32, kind="ExternalInput")
    woT = nc.dram_tensor("woT", (DIM, HD), BF16, kind="ExternalInput")
    bo = nc.dram_tensor("bo", (HD,), F32, kind="ExternalInput")
    outT = nc.dram_tensor("outT", (HD, N), BF16, kind="ExternalOutput")

    # per-head collectives: 256KB AllGathers pipeline best on this fabric
    # (bigger ops measured >2x slower per byte)
    agin = [nc.dram_tensor(f"agin{h}", (64, N), BF16) for h in range(HPC)]
    agout = [nc.dram_tensor(f"agout{h}", (256, N), BF16) for h in range(HPC)]

    with ExitStack() as ctx:
        tc = ctx.enter_context(tile.TileContext(nc))
        const = ctx.enter_context(tc.tile_pool(name="const", bufs=1))

        nc.gpsimd.load_library(library_config.attn)

        ident_f32 = const.tile([128, 128], F32)
        make_identity(nc, ident_f32)
        ident = const.tile([128, 128], BF16)
        nc.vector.tensor_copy(ident, ident_f32)

        pb_sb = const.tile([128, 6], F32)
        nc.sync.dma_start(out=pb_sb, in_=pb[:].rearrange("(i p) -> p i", p=128))
        aug_sb = const.tile([128, NVT], F32)
        nc.sync.dma_start(out=aug_sb, in_=aug[:].rearrange("(t p) -> p t", p=128))
        bo_sb = const.tile([128, HD // 128], F32)
        woT_sb = const.tile([128, KT, HD], BF16)
        woT_r = woT[:, :].rearrange("(k p) c -> p k c", p=128)

        qkvT = const.tile([128, 4, N], BF16)            # [q0 q1 | v0 v1]
        # k stationaries padded to 128 rows (zeros kill the paired head's q
        # contribution) and v stationaries padded to 128 cols — both enable
        # the PE fast-weight-load path (needs 128-row/col stationary).
        kTp = const.tile([128, HPC, NV], BF16)
        vsb = const.tile([128, NVT, HPC, 128], BF16)    # v[m,d] | aug | zeros
        att_sb = const.tile([128, 2 * HPC, N], BF16)    # gathered heads

        # warm the PE clock gate while input DMAs land (results unused);
        # 512-wide matmuls give a stronger activity signal than transposes
        wrhs = const.tile([128, 512], BF16)
        nc.vector.memset(wrhs, 0.0)
        with tc.tile_pool(name="pp_w", bufs=1, space="PSUM") as ppw:
            wt = ppw.tile([128, 512], F32, tag="warm")
            for _ in range(36):
                nc.tensor.matmul(wt, lhsT=ident, rhs=wrhs,
                                 start=True, stop=True)

        # ---- phase 1: fused qkv projection + v transpose/mask ----
        with tc.tile_pool(name="xw", bufs=1) as xw, \
             tc.tile_pool(name="pp_proj", bufs=6, space="PSUM") as ppp, \
             tc.tile_pool(name="pp_vt", bufs=2, space="PSUM") as ppvt:
            wT_sb = xw.tile([128, KT, 3 * HD], BF16)
            xT_sb = xw.tile([128, KT, N], BF16)
            xvT_sb = xw.tile([128, KT, NV], BF16)
            wT_r = wT[:, :].rearrange("(k p) m -> p k m", p=128)
            xT_r = xT[:, :].rearrange("(k p) n -> p k n", p=128)
            xvT_r = xvT[:, :].rearrange("(k p) n -> p k n", p=128)
            # DMA order matches consumption: wT+xT half 0 (q first half),
            # then xT half 1, then xvT (k/v), then late consts
            for k in range(KT):
                nc.sync.dma_start(out=wT_sb[:, k, :], in_=wT_r[:, k, :])
                nc.sync.dma_start(out=xT_sb[:, k, 0:N // 2],
                                  in_=xT_r[:, k, 0:N // 2])
            for k in range(KT):
                nc.sync.dma_start(out=xT_sb[:, k, N // 2:N],
                                  in_=xT_r[:, k, N // 2:N])
            for k in range(KT):
                nc.sync.dma_start(out=xvT_sb[:, k, :], in_=xvT_r[:, k, :])
            nc.sync.dma_start(out=bo_sb,
                              in_=bo[:].rearrange("(c p) -> p c", p=128))
            for k in range(KT):
                nc.sync.dma_start(out=woT_sb[:, k, :], in_=woT_r[:, k, :])

            # q rowtiles (i=0,1), k-outer per n-half: accumulation is paced
            # by the xT DMA stream instead of waiting for the last ktile
            for half in range(2):
                pss = [ppp.tile([128, 512], F32, tag="ps",
                                name=f"psq{half}_{c}") for c in range(4)]
                for k in range(KT):
                    for i in range(2):
                        lhs = wT_sb[:, k, i * 128:(i + 1) * 128]
                        for cc in range(2):
                            c = half * 2 + cc
                            nc.tensor.matmul(
                                pss[i * 2 + cc], lhsT=lhs,
                                rhs=xT_sb[:, k, c * 512:(c + 1) * 512],
                                start=(k == 0), stop=(k == KT - 1),
                            )
                for i in range(2):
                    for cc in range(2):
                        c = half * 2 + cc
                        nc.vector.tensor_scalar_add(
                            qkvT[:, i, c * 512:(c + 1) * 512],
                            pss[i * 2 + cc], pb_sb[:, i:i + 1],
                        )

            # zero pads once (disjoint partition/col ranges from the writes).
            # head h's k rows sit in the partition half matching its q rows
            # (h%2), the other half is zeros.
            for h in range(HPC):
                z0 = 64 if h % 2 == 0 else 0
                nc.vector.memset(kTp[z0:z0 + 64, h, :], 0.0)
            nc.vector.memset(vsb[:, :, :, D + 1:128], 0.0)

            # k/v rowtile pairs over NV gathered tokens, k-outer
            vchunks = [(s, min(512, NV - s)) for s in range(0, NV, 512)]
            for pair in range(2):       # 0: k rowtiles (2,3), 1: v (4,5)
                pss = [ppp.tile([128, 512], F32, tag="ps",
                                name=f"pskv{pair}_{t}")
                       for t in range(2 * len(vchunks))]
                for k in range(KT):
                    for j in range(2):
                        i = 2 + pair * 2 + j
                        lhs = wT_sb[:, k, i * 128:(i + 1) * 128]
                        for c, (s, w) in enumerate(vchunks):
                            nc.tensor.matmul(
                                pss[j * len(vchunks) + c][:, 0:w], lhsT=lhs,
                                rhs=xvT_sb[:, k, s:s + w],
                                start=(k == 0), stop=(k == KT - 1),
                            )
                for j in range(2):
                    i = 2 + pair * 2 + j
                    for c, (s, w) in enumerate(vchunks):
                        ps = pss[j * len(vchunks) + c]
                        if pair == 0:
                            # k rows (no bias) into kTp, each head's rows in
                            # its own partition half
                            for jj in range(2):
                                nc.vector.tensor_copy(
                                    kTp[jj * 64:jj * 64 + 64, 2 * j + jj,
                                        s:s + w],
                                    ps[jj * 64:jj * 64 + 64, 0:w],
                                )
                        else:
                            nc.vector.tensor_scalar_add(
                                qkvT[:, 2 + j, s:s + w], ps[:, 0:w],
                                pb_sb[:, i:i + 1],
                            )

            # transpose vT -> v[m, d]; zero pad rows; aug ones column
            for vi in range(2):
                for t in range(NVT):
                    vt = ppvt.tile([128, 128], BF16, tag="vt",
                                   name=f"vt{vi}_{t}")
                    nc.tensor.transpose(
                        vt, qkvT[:, 2 + vi, t * 128:(t + 1) * 128], ident
                    )
                    for hh in range(2):
                        h = vi * 2 + hh
                        nc.vector.tensor_scalar_mul(
                            vsb[:, t, h, 0:D], vt[:, hh * 64:hh * 64 + 64],
                            aug_sb[:, t:t + 1],
                        )
            for h in range(HPC):
                for t in range(NVT):
                    nc.vector.tensor_copy(vsb[:, t, h, D:D + 1],
                                          aug_sb[:, t:t + 1])

        # ---- phase 2: attention per head, AllGather overlapped ----
        with tc.tile_pool(name="expool", bufs=4) as expool, \
             tc.tile_pool(name="attp", bufs=2) as attp, \
             tc.tile_pool(name="ocpp", bufs=2) as ocpp, \
             tc.tile_pool(name="dnp", bufs=2) as dnp, \
             tc.tile_pool(name="recp", bufs=2) as recp, \
             tc.tile_pool(name="recbp", bufs=2) as recbp, \
             tc.tile_pool(name="pp_o", bufs=1, space="PSUM") as ppo, \
             tc.tile_pool(name="pp_st", bufs=2, space="PSUM") as ppst:
            for lh in range(HPC):
                ih = lh // 2
                qTh = qkvT[:, ih, :]
                op = ppo.tile([128, N], F32, tag="op")
                for t in range(NVT):
                    lhs = kTp[:, lh, t * 128:(t + 1) * 128]
                    for nh in range(2):
                        st = ppst.tile([128, 1024], F32, tag="st",
                                       name=f"st{lh}_{t}_{nh}")
                        for cc in range(2):
                            nc.tensor.matmul(
                                st[:, cc * 512:(cc + 1) * 512], lhsT=lhs,
                                rhs=qTh[:, nh * 1024 + cc * 512:
                                        nh * 1024 + (cc + 1) * 512],
                                start=True, stop=True,
                            )
                        ex = expool.tile([128, 1024], BF16, tag="ex",
                                         name=f"ex{lh}_{t}_{nh}")
                        nc.scalar.activation(ex, st, EXP)
                        vlhs = vsb[:, t, lh, :]
                        for cc in range(2):
                            nc.tensor.matmul(
                                op[:, nh * 1024 + cc * 512:
                                   nh * 1024 + (cc + 1) * 512],
                                lhsT=vlhs,
                                rhs=ex[:, cc * 512:(cc + 1) * 512],
                                start=(t == 0), stop=(t == NVT - 1),
                            )
                # drain PSUM to SBUF first (ALL op readers up front so the
                # next head's PV gets the banks ASAP), then normalize per
                # n-half so the chain pipelines
                ocps, dns = [], []
                for nh in range(2):
                    sl = slice(nh * (N // 2), (nh + 1) * (N // 2))
                    ocp = ocpp.tile([D, N // 2], F32, tag="ocp",
                                    name=f"ocp{lh}_{nh}")
                    nc.vector.tensor_copy(ocp, op[0:D, sl])
                    dn = dnp.tile([1, N // 2], F32, tag="dn",
                                  name=f"dn{lh}_{nh}")
                    nc.vector.tensor_copy(dn, op[D:D + 1, sl])
                    ocps.append(ocp)
                    dns.append(dn)
                for nh in range(2):
                    sl = slice(nh * (N // 2), (nh + 1) * (N // 2))
                    rec = recp.tile([1, N // 2], F32, tag="rec",
                                    name=f"rec{lh}_{nh}")
                    nc.vector.reciprocal_approx_fast(out=rec, in_=dns[nh])
                    recb = recbp.tile([64, N // 2], F32, tag="recb",
                                      name=f"recb{lh}_{nh}")
                    nc.gpsimd.partition_broadcast(recb, rec)
                    att = attp.tile([64, N // 2], BF16, tag="att",
                                    name=f"att{lh}_{nh}")
                    nc.vector.tensor_mul(att, ocps[nh], recb)
                    nc.sync.dma_start(out=agin[lh][:, sl], in_=att)
                nc.gpsimd.collective_compute(
                    "AllGather",
                    mybir.AluOpType.bypass,
                    replica_groups=GROUPS,
                    ins=[agin[lh][:, :].opt()],
                    outs=[agout[lh][:, :].opt()],
                )
                nc.sync.dma_start(out=att_sb[:, 2 * lh, :],
                                  in_=agout[lh][0:128, :])
                nc.sync.dma_start(out=att_sb[:, 2 * lh + 1, :],
                                  in_=agout[lh][128:256, :])

        # ---- phase 3: output projection slice ----
        with tc.tile_pool(name="outp", bufs=2) as outp, \
             tc.tile_pool(name="pp_f", bufs=2, space="PSUM") as ppf:
            out_r = outT[:, :].rearrange("(c p) n -> p c n", p=128)
            fps = [ppf.tile([128, N], F32, tag="fp", name=f"fp{ct}")
                   for ct in range(2)]
            # kt 0-3 (first AllGather) accumulate early
            for kt in range(4):
                for ct in range(2):
                    lhs = woT_sb[:, kt, ct * 128:(ct + 1) * 128]
                    for c in range(NCH):
                        nc.tensor.matmul(
                            fps[ct][:, c * 512:(c + 1) * 512],
                            lhsT=lhs,
                            rhs=att_sb[:, kt, c * 512:(c + 1) * 512],
                            start=(kt == 0), stop=False,
                        )
            # kt 4-7 (second AllGather) finished per (ct, nh) so each
            # bias-add + store leaves as soon as its chunks are done
            for ct in range(2):
                for nh in range(2):
                    for cc in range(2):
                        c = nh * 2 + cc
                        for kt in range(4, KT):
                            nc.tensor.matmul(
                                fps[ct][:, c * 512:(c + 1) * 512],
                                lhsT=woT_sb[:, kt, ct * 128:(ct + 1) * 128],
                                rhs=att_sb[:, kt, c * 512:(c + 1) * 512],
                                start=False, stop=(kt == KT - 1),
                            )
                    sl = slice(nh * (N // 2), (nh + 1) * (N // 2))
                    ot = outp.tile([128, N // 2], BF16, tag="ot",
                                   name=f"ot{ct}_{nh}")
                    nc.vector.tensor_scalar_add(ot, fps[ct][:, sl],
                                                bo_sb[:, ct:ct + 1])
                    nc.sync.dma_start(out=out_r[:, ct, sl], in_=ot)

    nc.finalize()
    return nc


def _prep_core_inputs(inputs, c, idxs, NV):
    b, g = c // 4, c % 4
    rows = slice(g * HD, (g + 1) * HD)
    w_qkv = np.asarray(inputs["w_qkv"], np.float32)
    Wq = (w_qkv[0:H * D][rows]
          + np.asarray(inputs["wq_base"], np.float32)[rows]
          + LS * (np.asarray(inputs["wq_B"], np.float32)[rows]
                  @ np.asarray(inputs["wq_A"], np.float32))) * ATT
    Wk = w_qkv[H * D:2 * H * D][rows]
    Wv = (w_qkv[2 * H * D:3 * H * D][rows]
          + np.asarray(inputs["wv_base"], np.float32)[rows]
          + LS * (np.asarray(inputs["wv_B"], np.float32)[rows]
                  @ np.asarray(inputs["wv_A"], np.float32)))
    wTv = np.ascontiguousarray(
        np.concatenate([Wq, Wk, Wv], 0).T.astype(BF))
    pbv = np.concatenate([
        np.asarray(inputs["bq_base"], np.float32)[rows] * ATT,
        np.zeros(HD, np.float32),
        np.asarray(inputs["bv_base"], np.float32)[rows],
    ]).astype(np.float32)

    x = np.asarray(inputs["x"], np.float32)[b]            # [N, DIM]
    idx = idxs[b]
    nv = len(idx)
    xv = np.zeros((NV, DIM), np.float32)
    xv[:nv] = x[idx]
    augv = np.zeros(NV, np.float32)
    augv[:nv] = 1.0
    xTv = np.ascontiguousarray(x.T.astype(BF))
    xvTv = np.ascontiguousarray(xv.T.astype(BF))

    # out-proj columns permuted to match AllGather row order:
    # hd' = lh*256 + j*64 + d  <->  global head 4j+lh, dim d
    perm = np.empty(H * D, np.int64)
    for i in range(H * D):
        lh, r = divmod(i, 4 * D)
        j, d = divmod(r, D)
        perm[i] = (4 * j + lh) * D + d
    w_out = np.asarray(inputs["w_out"], np.float32)
    woTv = np.ascontiguousarray(w_out[rows, :][:, perm].T.astype(BF))
    bov = np.asarray(inputs["b_out"], np.float32)[rows]
    return {"xT": xTv, "xvT": xvTv, "wT": wTv, "pb": pbv, "aug": augv,
            "woT": woTv, "bo": bov}


def kernel(**inputs):
    global LAST_RESULTS
    mask = np.asarray(inputs["mask"]).astype(bool)
    idxs = [np.flatnonzero(mask[b]) for b in range(B)]
    NV = max(128, -(-max(len(ix) for ix in idxs) // 128) * 128)
    if NV not in _NC_CACHE:
        _NC_CACHE[NV] = _build_nc(NV)
    nc = _NC_CACHE[NV]
    in_maps = [_prep_core_inputs(inputs, c, idxs, NV) for c in range(NCORES)]
    res = bass_utils.run_bass_kernel_spmd(
        nc, in_maps, core_ids=list(range(NCORES)),
        trace=TRACE, tmpdir=TRACE_DIR,
    )
    LAST_RESULTS = res
    out = np.empty((B, N, DIM), np.float32)
    for c in range(NCORES):
        b, g = c // 4, c % 4
        out[b, :, g * HD:(g + 1) * HD] = \
            np.asarray(res.results[c]["outT"]).astype(np.float32).T
    return out


# revision 40
# speedup vs baseline: 1.1793x; 1.0193x over previous
"""LoRA attention kernel for 8 Trainium2 NeuronCores.

Sharding: data-parallel over batch B=2 (cores 0-3 -> b=0, cores 4-7 -> b=1),
tensor-parallel over heads within each batch group (4 heads/core). LoRA paths
and q/v base linears are folded host-side into one effective qkv weight.

Key optimizations over the fp32r baseline:
- All matmuls in bf16: fp32r never registers activity with the PE's HAM
  clock gate, so the array sat at K=4/8 (1.2 GHz) for ~485us of the run and
  paid full-rate LDWEIGHTS (no FWL). bf16 runs warm at 2.4 GHz with fast
  weight load.
- Key-padding mask applied by gathering valid tokens host-side: k/v
  projections, scores, exp and P@V run over ~Nv~1024 instead of 2048 keys.
- Softmax denominator via an augmented ones-column in the P@V matmul;
  reciprocal via the fast custom DVE op on [1,N) + gpsimd partition
  broadcast (baseline burned 52us in single-lane DVE reciprocals plus a
  DRAM round-trip broadcast).
- Per-head bf16 AllGathers of attention outputs overlap the next head's
  compute (baseline: one 103us fp32 AllGather dead on the tail).
"""

import sys
from contextlib import ExitStack

import numpy as np
import ml_dtypes

for _p in ("/opt/trn_rl_repo", "/opt/trn_rl_repo/concourse"):
    if _p not in sys.path:
        sys.path.insert(0, _p)

import concourse.bass as bass
import concourse.mybir as mybir
import concourse.tile as tile
from concourse import bacc
from concourse import bass_utils
from concourse import library_config
from concourse.masks import make_identity

F32 = mybir.dt.float32
BF16 = mybir.dt.bfloat16
EXP = mybir.ActivationFunctionType.Exp
BF = ml_dtypes.bfloat16

H, D, DIM, R = 16, 64, 1024, 10
B, N = 2, 2048
NCORES = 8
GROUPS = [[0, 1, 2, 3], [4, 5, 6, 7]]
HPC = H // 4          # heads per core
HD = HPC * D          # 256 qkv rows per core per projection
ATT = float(D) ** -0.5
LS = 1.0 / R

KT = DIM // 128       # 8 contraction tiles
NCH = N // 512        # 4 moving chunks of 512

# test harness hooks
TRACE = False
TRACE_DIR = None
LAST_RESULTS = None

_NC_CACHE = {}


def _build_nc(NV):
    """Build the SPMD program for NV (padded valid-key count, mult of 128)."""
    NVT = NV // 128                      # m-tiles over valid keys
    nc = bacc.Bacc(None, target_bir_lowering=False, num_devices=NCORES)

    xT = nc.dram_tensor("xT", (DIM, N), BF16, kind="ExternalInput")
    xvT = nc.dram_tensor("xvT", (DIM, NV), BF16, kind="ExternalInput")
    wT = nc.dram_tensor("wT", (DIM, 3 * HD), BF16, kind="ExternalInput")
    pb = nc.dram_tensor("pb", (3 * HD,), F32, kind="ExternalInput")
    aug = nc.dram_tensor("aug", (NV,), F32, kind="ExternalInput")
    woT = nc.dram_tensor("woT", (DIM, HD), BF16, kind="ExternalInput")
    bo = nc.dram_tensor("bo", (HD,), F32, kind="ExternalInput")
    outT = nc.dram_tensor("outT", (HD, N), BF16, kind="ExternalOutput")

    # per-head collectives: 256KB AllGathers pipeline best on this fabric
    # (bigger ops measured >2x slower per byte)
    agin = [nc.dram_tensor(f"agin{h}", (64, N), BF16) for h in range(HPC)]
    agout = [nc.dram_tensor(f"agout{h}", (256, N), BF16) for h in range(HPC)]
    # tiny warmup collective: absorbs the collective-stream barrier and
    # first-op firmware cost at t~0 instead of delaying the real chain
    agwi = nc.dram_tensor("agwi", (1, 128), BF16)
    agwo = nc.dram_tensor("agwo", (4, 128), BF16)

    with ExitStack() as ctx:
        tc = ctx.enter_context(tile.TileContext(nc))
        const = ctx.enter_context(tc.tile_pool(name="const", bufs=1))

        nc.gpsimd.load_library(library_config.attn)

        nc.gpsimd.collective_compute(
            "AllGather",
            mybir.AluOpType.bypass,
            replica_groups=GROUPS,
            ins=[agwi[:, :].opt()],
            outs=[agwo[:, :].opt()],
        )

        ident_f32 = const.tile([128, 128], F32)
        make_identity(nc, ident_f32)
        ident = const.tile([128, 128], BF16)
        nc.vector.tensor_copy(ident, ident_f32)

        pb_sb = const.tile([128, 6], F32)
        nc.sync.dma_start(out=pb_sb, in_=pb[:].rearrange("(i p) -> p i", p=128))
        aug_sb = const.tile([128, NVT], F32)
        nc.sync.dma_start(out=aug_sb, in_=aug[:].rearrange("(t p) -> p t", p=128))
        bo_sb = const.tile([128, HD // 128], F32)
        woT_sb = const.tile([128, KT, HD], BF16)
        woT_r = woT[:, :].rearrange("(k p) c -> p k c", p=128)

        qkvT = const.tile([128, 4, N], BF16)            # [q0 q1 | v0 v1]
        # k stationaries padded to 128 rows (zeros kill the paired head's q
        # contribution) and v stationaries padded to 128 cols — both enable
        # the PE fast-weight-load path (needs 128-row/col stationary).
        kTp = const.tile([128, HPC, NV], BF16)
        vsb = const.tile([128, NVT, HPC, 128], BF16)    # v[m,d] | aug | zeros
        att_sb = const.tile([128, 2 * HPC, N], BF16)    # gathered heads

        # warm the PE clock gate while input DMAs land (results unused);
        # 512-wide matmuls give a stronger activity signal than transposes
        wrhs = const.tile([128, 512], BF16)
        nc.vector.memset(wrhs, 0.0)
        with tc.tile_pool(name="pp_w", bufs=1, space="PSUM") as ppw:
            wt = ppw.tile([128, 512], F32, tag="warm")
            for _ in range(36):
                nc.tensor.matmul(wt, lhsT=ident, rhs=wrhs,
                                 start=True, stop=True)

        # ---- phase 1: fused qkv projection + v transpose/mask ----
        with tc.tile_pool(name="xw", bufs=1) as xw, \
             tc.tile_pool(name="pp_proj", bufs=6, space="PSUM") as ppp, \
             tc.tile_pool(name="pp_vt", bufs=2, space="PSUM") as ppvt:
            wT_sb = xw.tile([128, KT, 3 * HD], BF16)
            xT_sb = xw.tile([128, KT, N], BF16)
            xvT_sb = xw.tile([128, KT, NV], BF16)
            wT_r = wT[:, :].rearrange("(k p) m -> p k m", p=128)
            xT_r = xT[:, :].rearrange("(k p) n -> p k n", p=128)
            xvT_r = xvT[:, :].rearrange("(k p) n -> p k n", p=128)
            # DMA order matches consumption: wT+xT half 0 (q first half),
            # then xT half 1, then xvT (k/v), then late consts
            for k in range(KT):
                nc.sync.dma_start(out=wT_sb[:, k, :], in_=wT_r[:, k, :])
                nc.sync.dma_start(out=xT_sb[:, k, 0:N // 2],
                                  in_=xT_r[:, k, 0:N // 2])
            for k in range(KT):
                nc.sync.dma_start(out=xT_sb[:, k, N // 2:N],
                                  in_=xT_r[:, k, N // 2:N])
            for k in range(KT):
                nc.sync.dma_start(out=xvT_sb[:, k, :], in_=xvT_r[:, k, :])
            nc.sync.dma_start(out=bo_sb,
                              in_=bo[:].rearrange("(c p) -> p c", p=128))
            for k in range(KT):
                nc.sync.dma_start(out=woT_sb[:, k, :], in_=woT_r[:, k, :])

            # q rowtiles (i=0,1), k-outer per n-half: accumulation is paced
            # by the xT DMA stream instead of waiting for the last ktile
            for half in range(2):
                pss = [ppp.tile([128, 512], F32, tag="ps",
                                name=f"psq{half}_{c}") for c in range(4)]
                for k in range(KT):
                    for i in range(2):
                        lhs = wT_sb[:, k, i * 128:(i + 1) * 128]
                        for cc in range(2):
                            c = half * 2 + cc
                            nc.tensor.matmul(
                                pss[i * 2 + cc], lhsT=lhs,
                                rhs=xT_sb[:, k, c * 512:(c + 1) * 512],
                                start=(k == 0), stop=(k == KT - 1),
                            )
                for i in range(2):
                    for cc in range(2):
                        c = half * 2 + cc
                        nc.vector.tensor_scalar_add(
                            qkvT[:, i, c * 512:(c + 1) * 512],
                            pss[i * 2 + cc], pb_sb[:, i:i + 1],
                        )

            # zero pads once (disjoint partition/col ranges from the writes).
            # head h's k rows sit in the partition half matching its q rows
            # (h%2), the other half is zeros.
            for h in range(HPC):
                z0 = 64 if h % 2 == 0 else 0
                nc.vector.memset(kTp[z0:z0 + 64, h, :], 0.0)
            nc.vector.memset(vsb[:, :, :, D + 1:128], 0.0)

            # k/v rowtile pairs over NV gathered tokens, k-outer
            vchunks = [(s, min(512, NV - s)) for s in range(0, NV, 512)]
            for pair in range(2):       # 0: k rowtiles (2,3), 1: v (4,5)
                pss = [ppp.tile([128, 512], F32, tag="ps",
                                name=f"pskv{pair}_{t}")
                       for t in range(2 * len(vchunks))]
                for k in range(KT):
                    for j in range(2):
                        i = 2 + pair * 2 + j
                        lhs = wT_sb[:, k, i * 128:(i + 1) * 128]
                        for c, (s, w) in enumerate(vchunks):
                            nc.tensor.matmul(
                                pss[j * len(vchunks) + c][:, 0:w], lhsT=lhs,
                                rhs=xvT_sb[:, k, s:s + w],
                                start=(k == 0), stop=(k == KT - 1),
                            )
                for j in range(2):
                    i = 2 + pair * 2 + j
                    for c, (s, w) in enumerate(vchunks):
                        ps = pss[j * len(vchunks) + c]
                        if pair == 0:
                            # k rows (no bias) into kTp, each head's rows in
                            # its own partition half
                            for jj in range(2):
                                nc.vector.tensor_copy(
                                    kTp[jj * 64:jj * 64 + 64, 2 * j + jj,
                                        s:s + w],
                                    ps[jj * 64:jj * 64 + 64, 0:w],
                                )
                        else:
                            nc.vector.tensor_scalar_add(
                                qkvT[:, 2 + j, s:s + w], ps[:, 0:w],
                                pb_sb[:, i:i + 1],
                            )

            # transpose vT -> v[m, d]; zero pad rows; aug ones column
            for vi in range(2):
                for t in range(NVT):
                    vt = ppvt.tile([128, 128], BF16, tag="vt",
                                   name=f"vt{vi}_{t}")
                    nc.tensor.transpose(
                        vt, qkvT[:, 2 + vi, t * 128:(t + 1) * 128], ident
                    )
                    for hh in range(2):
                        h = vi * 2 + hh
                        nc.vector.tensor_scalar_mul(
                            vsb[:, t, h, 0:D], vt[:, hh * 64:hh * 64 + 64],
                            aug_sb[:, t:t + 1],
                        )
            for h in range(HPC):
                for t in range(NVT):
                    nc.vector.tensor_copy(vsb[:, t, h, D:D + 1],
                                          aug_sb[:, t:t + 1])

        # ---- phase 2: attention per head, AllGather overlapped ----
        with tc.tile_pool(name="expool", bufs=6) as expool, \
             tc.tile_pool(name="attp", bufs=2) as attp, \
             tc.tile_pool(name="ocpp", bufs=2) as ocpp, \
             tc.tile_pool(name="dnp", bufs=2) as dnp, \
             tc.tile_pool(name="recp", bufs=2) as recp, \
             tc.tile_pool(name="recbp", bufs=2) as recbp, \
             tc.tile_pool(name="pp_o", bufs=1, space="PSUM") as ppo, \
             tc.tile_pool(name="pp_st", bufs=2, space="PSUM") as ppst:
            for lh in range(HPC):
                ih = lh // 2
                qTh = qkvT[:, ih, :]
                op = ppo.tile([128, N], F32, tag="op")
                for t in range(NVT):
                    lhs = kTp[:, lh, t * 128:(t + 1) * 128]
                    for nh in range(2):
                        st = ppst.tile([128, 1024], F32, tag="st",
                                       name=f"st{lh}_{t}_{nh}")
                        for cc in range(2):
                            nc.tensor.matmul(
                                st[:, cc * 512:(cc + 1) * 512], lhsT=lhs,
                                rhs=qTh[:, nh * 1024 + cc * 512:
                                        nh * 1024 + (cc + 1) * 512],
                                start=True, stop=True,
                            )
                        ex = expool.tile([128, 1024], BF16, tag="ex",
                                         name=f"ex{lh}_{t}_{nh}")
                        nc.scalar.activation(ex, st, EXP)
                        vlhs = vsb[:, t, lh, :]
                        for cc in range(2):
                            nc.tensor.matmul(
                                op[:, nh * 1024 + cc * 512:
                                   nh * 1024 + (cc + 1) * 512],
                                lhsT=vlhs,
                                rhs=ex[:, cc * 512:(cc + 1) * 512],
                                start=(t == 0), stop=(t == NVT - 1),
                            )
                # drain PSUM to SBUF first (ALL op readers up front so the
                # next head's PV gets the banks ASAP), then normalize per
                # n-half so the chain pipelines
                ocps, dns = [], []
                for nh in range(2):
                    sl = slice(nh * (N // 2), (nh + 1) * (N // 2))
                    ocp = ocpp.tile([D, N // 2], F32, tag="ocp",
                                    name=f"ocp{lh}_{nh}")
                    nc.vector.tensor_copy(ocp, op[0:D, sl])
                    dn = dnp.tile([1, N // 2], F32, tag="dn",
                                  name=f"dn{lh}_{nh}")
                    nc.vector.tensor_copy(dn, op[D:D + 1, sl])
                    ocps.append(ocp)
                    dns.append(dn)
                for nh in range(2):
                    sl = slice(nh * (N // 2), (nh + 1) * (N // 2))
                    rec = recp.tile([1, N // 2], F32, tag="rec",
                                    name=f"rec{lh}_{nh}")
                    nc.vector.reciprocal_approx_fast(out=rec, in_=dns[nh])
                    recb = recbp.tile([64, N // 2], F32, tag="recb",
                                      name=f"recb{lh}_{nh}")
                    nc.gpsimd.partition_broadcast(recb, rec)
                    att = attp.tile([64, N // 2], BF16, tag="att",
                                    name=f"att{lh}_{nh}")
                    nc.vector.tensor_mul(att, ocps[nh], recb)
                    nc.sync.dma_start(out=agin[lh][:, sl], in_=att)
                nc.gpsimd.collective_compute(
                    "AllGather",
                    mybir.AluOpType.bypass,
                    replica_groups=GROUPS,
                    ins=[agin[lh][:, :].opt()],
                    outs=[agout[lh][:, :].opt()],
                )
                nc.sync.dma_start(out=att_sb[:, 2 * lh, :],
                                  in_=agout[lh][0:128, :])
                nc.sync.dma_start(out=att_sb[:, 2 * lh + 1, :],
                                  in_=agout[lh][128:256, :])

        # ---- phase 3: output projection slice ----
        with tc.tile_pool(name="outp", bufs=2) as outp, \
             tc.tile_pool(name="pp_f", bufs=2, space="PSUM") as ppf:
            out_r = outT[:, :].rearrange("(c p) n -> p c n", p=128)
            fps = [ppf.tile([128, N], F32, tag="fp", name=f"fp{ct}")
                   for ct in range(2)]
            # kt 0-3 (first AllGather) accumulate early
            for kt in range(4):
                for ct in range(2):
                    lhs = woT_sb[:, kt, ct * 128:(ct + 1) * 128]
                    for c in range(NCH):
                        nc.tensor.matmul(
                            fps[ct][:, c * 512:(c + 1) * 512],
                            lhsT=lhs,
                            rhs=att_sb[:, kt, c * 512:(c + 1) * 512],
                            start=(kt == 0), stop=False,
                        )
            # kt 4-7 (second AllGather) finished per (ct, nh) so each
            # bias-add + store leaves as soon as its chunks are done
            for ct in range(2):
                for nh in range(2):
                    for cc in range(2):
                        c = nh * 2 + cc
                        for kt in range(4, KT):
                            nc.tensor.matmul(
                                fps[ct][:, c * 512:(c + 1) * 512],
                                lhsT=woT_sb[:, kt, ct * 128:(ct + 1) * 128],
                                rhs=att_sb[:, kt, c * 512:(c + 1) * 512],
                                start=False, stop=(kt == KT - 1),
                            )
                    sl = slice(nh * (N // 2), (nh + 1) * (N // 2))
                    ot = outp.tile([128, N // 2], BF16, tag="ot",
                                   name=f"ot{ct}_{nh}")
                    nc.vector.tensor_scalar_add(ot, fps[ct][:, sl],
                                                bo_sb[:, ct:ct + 1])
                    nc.sync.dma_start(out=out_r[:, ct, sl], in_=ot)

    nc.finalize()
    return nc


def _prep_core_inputs(inputs, c, idxs, NV):
    b, g = c // 4, c % 4
    rows = slice(g * HD, (g + 1) * HD)
    w_qkv = np.asarray(inputs["w_qkv"], np.float32)
    Wq = (w_qkv[0:H * D][rows]
          + np.asarray(inputs["wq_base"], np.float32)[rows]
          + LS * (np.asarray(inputs["wq_B"], np.float32)[rows]
                  @ np.asarray(inputs["wq_A"], np.float32))) * ATT
    Wk = w_qkv[H * D:2 * H * D][rows]
    Wv = (w_qkv[2 * H * D:3 * H * D][rows]
          + np.asarray(inputs["wv_base"], np.float32)[rows]
          + LS * (np.asarray(inputs["wv_B"], np.float32)[rows]
                  @ np.asarray(inputs["wv_A"], np.float32)))
    wTv = np.ascontiguousarray(
        np.concatenate([Wq, Wk, Wv], 0).T.astype(BF))
    pbv = np.concatenate([
        np.asarray(inputs["bq_base"], np.float32)[rows] * ATT,
        np.zeros(HD, np.float32),
        np.asarray(inputs["bv_base"], np.float32)[rows],
    ]).astype(np.float32)

    x = np.asarray(inputs["x"], np.float32)[b]            # [N, DIM]
    idx = idxs[b]
    nv = len(idx)
    xv = np.zeros((NV, DIM), np.float32)
    xv[:nv] = x[idx]
    augv = np.zeros(NV, np.float32)
    augv[:nv] = 1.0
    xTv = np.ascontiguousarray(x.T.astype(BF))
    xvTv = np.ascontiguousarray(xv.T.astype(BF))

    # out-proj columns permuted to match AllGather row order:
    # hd' = lh*256 + j*64 + d  <->  global head 4j+lh, dim d
    perm = np.empty(H * D, np.int64)
    for i in range(H * D):
        lh, r = divmod(i, 4 * D)
        j, d = divmod(r, D)
        perm[i] = (4 * j + lh) * D + d
    w_out = np.asarray(inputs["w_out"], np.float32)
    woTv = np.ascontiguousarray(w_out[rows, :][:, perm].T.astype(BF))
    bov = np.asarray(inputs["b_out"], np.float32)[rows]
    return {"xT": xTv, "xvT": xvTv, "wT": wTv, "pb": pbv, "aug": augv,
            "woT": woTv, "bo": bov}


def kernel(**inputs):
    global LAST_RESULTS
    mask = np.asarray(inputs["mask"]).astype(bool)
    idxs = [np.flatnonzero(mask[b]) for b in range(B)]
    NV = max(128, -(-max(len(ix) for ix in idxs) // 128) * 128)
    if NV not in _NC_CACHE:
        _NC_CACHE[NV] = _build_nc(NV)
    nc = _NC_CACHE[NV]
    in_maps = [_prep_core_inputs(inputs, c, idxs, NV) for c in range(NCORES)]
    res = bass_utils.run_bass_kernel_spmd(
        nc, in_maps, core_ids=list(range(NCORES)),
        trace=TRACE, tmpdir=TRACE_DIR,
    )
    LAST_RESULTS = res
    out = np.empty((B, N, DIM), np.float32)
    for c in range(NCORES):
        b, g = c // 4, c % 4
        out[b, :, g * HD:(g + 1) * HD] = \
            np.asarray(res.results[c]["outT"]).astype(np.float32).T
    return out
